# revision 19
# baseline (speedup 1.0000x reference)
"""Trainium2 Bass kernel for nn_GatedElementBasedRNNLayer_Loop.

Structure (per core, batch shard b=16 of B=128 across 8 cores):
  - init: load inputs, project passage/question through WuP/WuQ on PE,
    lay question out as [(q8,b) partitions, (qi,h) free] (Q=50 padded to 56,
    pad rows get -30*sign(v) so their softmax weight is ~e^-9: negligible).
  - 400 sequential GRU-with-attention steps, all in bf16 (f32 state/psum):
      attention pre-act built in PSUM by PE (question feed + pl broadcast),
      tanh on ACT, v-dot via 7x tensor_tensor_reduce on DVE, softmax sums
      via ACT accum + a 0/1-matrix matmul, ct via masked-diagonal matmuls,
      GRU matmuls with small stationaries, sigmoid as 0.5*tanh(0.5x)+0.5
      (keeps ACT on one table set: exp+tanh).
"""
import sys
import numpy as np

sys.path.insert(0, "/opt/trn_rl_repo")

P, Q, B, I, H = 400, 50, 128, 256, 128
NC_ = 8
BS = B // NC_          # 16 per-core batch
Q8, QI = 8, 7          # q = qi*8 + q8, Q padded to 56
QPAD = Q8 * QI         # 56

_compiled = None


def _build_nc(n_steps=P, dbg_tap=None):
    import concourse.bass as bass
    import concourse.bacc as bacc
    from concourse import tile
    import concourse.mybir as mybir

    dt = mybir.dt
    f32, bf16 = dt.float32, dt.bfloat16
    Alu = mybir.AluOpType
    Act = mybir.ActivationFunctionType

    nc = bacc.Bacc("TRN2", target_bir_lowering=False, debug=False, num_devices=NC_)

    # ---- DRAM parameters (per-core) ----
    pr_d = nc.dram_tensor("pr", [n_steps, BS, I], f32, kind="ExternalInput")
    qr_d = nc.dram_tensor("qr", [Q, BS, I], f32, kind="ExternalInput")
    wupt_d = nc.dram_tensor("wupt", [2, 128, H], bf16, kind="ExternalInput")
    wuqt_d = nc.dram_tensor("wuqt", [2, 128, H], bf16, kind="ExternalInput")
    wvpt_d = nc.dram_tensor("wvpt", [H, H], bf16, kind="ExternalInput")
    wgt_d = nc.dram_tensor("wgt", [2, 128, 2 * H], bf16, kind="ExternalInput")
    wiht_d = nc.dram_tensor("wiht", [2, 128, 3 * H], bf16, kind="ExternalInput")
    whht_d = nc.dram_tensor("whht", [H, 3 * H], bf16, kind="ExternalInput")
    i128_d = nc.dram_tensor("i128", [128, 128], bf16, kind="ExternalInput")
    i16_d = nc.dram_tensor("i16", [16, 16], bf16, kind="ExternalInput")
    i16f_d = nc.dram_tensor("i16f", [16, 16], f32, kind="ExternalInput")
    rmat_d = nc.dram_tensor("rmat", [16, 128], bf16, kind="ExternalInput")
    gmat_d = nc.dram_tensor("gmat", [128, 128], f32, kind="ExternalInput")
    bsel_d = nc.dram_tensor("bsel", [128, 16], bf16, kind="ExternalInput")
    mask_d = nc.dram_tensor("mask", [128, QI * BS], bf16, kind="ExternalInput")
    vrep_d = nc.dram_tensor("vrep", [128, H], bf16, kind="ExternalInput")
    padq_d = nc.dram_tensor("padq", [96, H], bf16, kind="ExternalInput")
    out_d = nc.dram_tensor("out", [n_steps, BS, H], f32, kind="ExternalOutput")
    _tap_shapes = {
        "pl": [16, H], "s": [128, QI * H], "logits": [128, QI], "ai": [128, QI],
        "ct": [16, H], "tg": [128, 32], "rzaff": [16, 256], "nt": [16, H],
        "ediag": [128, QI * BS], "uTp": [128, 16], "uTc": [128, 16], "girz": [16, 256], "girz_nogh": [16, 256],
    }
    dbg_d = None
    if dbg_tap is not None:
        dbg_d = nc.dram_tensor(
            "dbg", [n_steps] + _tap_shapes[dbg_tap], f32, kind="ExternalOutput"
        )

    with tile.TileContext(nc) as tc:
        with (
            tc.tile_pool(name="const", bufs=1) as cpool,
            tc.tile_pool(name="work", bufs=2) as wpool,
            tc.tile_pool(name="small", bufs=3) as spool,
            tc.tile_pool(name="ring", bufs=2) as rpool,
        ):
            # ---------------- constants to SBUF ----------------
            i128 = cpool.tile([128, 128], bf16, tag="i128")
            nc.sync.dma_start(i128[:, :], i128_d[:, :])
            i16 = cpool.tile([16, 16], bf16, tag="i16")
            nc.sync.dma_start(i16[:, :], i16_d[:, :])
            i16f = cpool.tile([16, 16], f32, tag="i16f")
            nc.sync.dma_start(i16f[:, :], i16f_d[:, :])
            rmat = cpool.tile([16, 128], bf16, tag="rmat")
            nc.sync.dma_start(rmat[:, :], rmat_d[:, :])
            gmat = cpool.tile([128, 128], f32, tag="gmat")
            nc.sync.dma_start(gmat[:, :], gmat_d[:, :])
            mask = cpool.tile([128, QI * BS], bf16, tag="mask")
            nc.sync.dma_start(mask[:, :], mask_d[:, :])
            vrep = cpool.tile([128, H], bf16, tag="vrep")
            nc.sync.dma_start(vrep[:, :], vrep_d[:, :])
            wvpt = cpool.tile([H, H], bf16, tag="wvpt")
            nc.sync.dma_start(wvpt[:, :], wvpt_d[:, :])
            whht = cpool.tile([H, 3 * H], bf16, tag="whht")
            nc.sync.dma_start(whht[:, :], whht_d[:, :])
            wgt = cpool.tile([128, 2 * 2 * H], bf16, tag="wgt")  # chunks side by side
            nc.sync.dma_start(wgt[:, 0 : 2 * H], wgt_d[0, :, :])
            nc.sync.dma_start(wgt[:, 2 * H : 4 * H], wgt_d[1, :, :])
            wiht = cpool.tile([128, 2 * 3 * H], bf16, tag="wiht")
            nc.sync.dma_start(wiht[:, 0 : 3 * H], wiht_d[0, :, :])
            nc.sync.dma_start(wiht[:, 3 * H : 6 * H], wiht_d[1, :, :])
            wupt = cpool.tile([128, 2 * H], bf16, tag="wupt")
            nc.sync.dma_start(wupt[:, 0:H], wupt_d[0, :, :])
            nc.sync.dma_start(wupt[:, H : 2 * H], wupt_d[1, :, :])
            wuqt = cpool.tile([128, 2 * H], bf16, tag="wuqt")
            nc.sync.dma_start(wuqt[:, 0:H], wuqt_d[0, :, :])
            nc.sync.dma_start(wuqt[:, H : 2 * H], wuqt_d[1, :, :])

            # persistent step data
            qB = cpool.tile([128, QI * H], bf16, tag="qB")          # question layout B'
            pprojT = cpool.tile([128, n_steps * BS], bf16, tag="pprojT")   # [h, (p,b)]

            # ---------------- input load + projections ----------------
            with (
                tc.tile_pool(name="init", bufs=1) as ipool,
                tc.tile_pool(name="ps_init", bufs=2, space="PSUM") as ps_ipool,
            ):
                # question: load, cast, project, transpose into qB
                qrf = ipool.tile([128, 2, Q * BS], f32, tag="qrf")
                for c in range(2):
                    nc.sync.dma_start(
                        qrf[:, c, :],
                        qr_d[:, :, 128 * c : 128 * (c + 1)].rearrange(
                            "q b i -> i (q b)"
                        ),
                    )
                qrb = ipool.tile([128, 2, Q * BS], bf16, tag="qrb")
                nc.vector.tensor_copy(qrb[:, :, :], qrf[:, :, :])
                for qb in range(QI):
                    ncols = min(8, Q - 8 * qb) * BS  # 128, last block 32
                    ps_q = ps_ipool.tile([128, 128], f32, tag="psq")
                    for c in range(2):
                        nc.tensor.matmul(
                            ps_q[:, 0:ncols],
                            wuqt[:, H * c : H * (c + 1)],
                            qrb[:, c, 8 * qb * BS : 8 * qb * BS + ncols],
                            start=(c == 0),
                            stop=(c == 1),
                        )
                    qp_sb = ipool.tile([128, 128], bf16, tag="qp_sb")
                    nc.scalar.copy(qp_sb[:, 0:ncols], ps_q[:, 0:ncols])
                    ps_qT = ps_ipool.tile([128, 128], bf16, tag="psqT")
                    nc.tensor.transpose(
                        ps_qT[0:ncols, :], qp_sb[:, 0:ncols], i128[:, :]
                    )
                    nc.scalar.copy(
                        qB[0:ncols, H * qb : H * (qb + 1)], ps_qT[0:ncols, :]
                    )
                # pad rows of last q-block
                nc.sync.dma_start(qB[32:128, H * 6 : H * 7], padq_d[:, :])

                # passage: process in column-chunks of 80 p to bound SBUF
                CH = min(80, n_steps)
                for chunk in range(n_steps // CH):
                    p0 = chunk * CH
                    prf = ipool.tile([128, 2, CH * BS], f32, tag="prf")
                    for c in range(2):
                        nc.sync.dma_start(
                            prf[:, c, :],
                            pr_d[p0 : p0 + CH, :, 128 * c : 128 * (c + 1)].rearrange(
                                "p b i -> i (p b)"
                            ),
                        )
                    prb = ipool.tile([128, 2, CH * BS], bf16, tag="prb")
                    nc.vector.tensor_copy(prb[:, :, :], prf[:, :, :])
                    for pb in range(CH // 8):
                        ps_p = ps_ipool.tile([128, 128], f32, tag="psp")
                        for c in range(2):
                            nc.tensor.matmul(
                                ps_p[:, :],
                                wupt[:, H * c : H * (c + 1)],
                                prb[:, c, 128 * pb : 128 * (pb + 1)],
                                start=(c == 0),
                                stop=(c == 1),
                            )
                        col = (p0 // 8 + pb) * 128
                        nc.scalar.copy(pprojT[:, col : col + 128], ps_p[:, :])

            # ---------------- state init ----------------
            ps_s_pool = tc.alloc_tile_pool(name="ps_s", bufs=1, space="PSUM")
            ps_pl_pool = tc.alloc_tile_pool(name="ps_pl", bufs=1, space="PSUM")
            ps_misc_pool = tc.alloc_tile_pool(name="ps_misc", bufs=2, space="PSUM")
            ps_ct_pool = tc.alloc_tile_pool(name="ps_ct", bufs=1, space="PSUM")
            ps_rz_pool = tc.alloc_tile_pool(name="ps_rz", bufs=1, space="PSUM")
            ps_gn_pool = tc.alloc_tile_pool(name="ps_gn", bufs=1, space="PSUM")
            prevT = spool.tile([128, 16], bf16, tag="prevT")
            nc.vector.memset(prevT[:, :], 0.0)
            prev_b = spool.tile([16, H], f32, tag="prev_b")
            nc.vector.memset(prev_b[:, :], 0.0)
            ring = rpool.tile([16, 8, H], f32, tag="ring")

            # ---------------- main loop ----------------
            for p in range(n_steps):
                a = p % 8
                pblk = p // 8
                # pl = prev @ WvP.T + pproj[p]  -> psum (16,128)
                ps_pl = ps_pl_pool.tile([16, H], f32, tag="ps_pl")
                nc.tensor.matmul(
                    ps_pl[:, :], prevT[:, :], wvpt[:, :], start=True, stop=False
                )
                nc.tensor.matmul(
                    ps_pl[:, :],
                    pprojT[:, p * BS : (p + 1) * BS],
                    i128[:, :],
                    start=False,
                    stop=True,
                )
                pl_sb = spool.tile([16, H], bf16, tag="pl_sb")
                nc.scalar.copy(pl_sb[:, :], ps_pl[:, :])

                # psum_s = question + broadcast(pl)
                ps_s = ps_s_pool.tile([128, QI * H], f32, tag="ps_s")
                nc.tensor.matmul(
                    ps_s[:, 0:512], i128[:, :], qB[:, 0:512],
                    start=True, stop=False, skip_group_check=True,
                )
                nc.tensor.matmul(
                    ps_s[:, 512:896], i128[:, :], qB[:, 512:896],
                    start=True, stop=False, skip_group_check=True,
                )
                for qi in range(QI):
                    nc.tensor.matmul(
                        ps_s[:, qi * H : (qi + 1) * H],
                        rmat[:, :],
                        pl_sb[:, :],
                        start=False,
                        stop=(qi == 3 or qi == 6),
                        skip_group_check=True,
                    )
                s_sb = wpool.tile([128, QI * H], bf16, tag="s_sb")
                nc.scalar.activation(s_sb[:, :], ps_s[:, :], Act.Tanh)

                # logits[(q8,b), qi] = sum_h s*v
                logits = spool.tile([128, QI], f32, tag="logits")
                scratch = wpool.tile([128, QI * H], bf16, tag="scratch")
                nc.vector.tensor_tensor(
                    scratch[:, :].rearrange("p (q h) -> p q h", q=QI),
                    s_sb[:, :].rearrange("p (q h) -> p q h", q=QI),
                    vrep[:, :].unsqueeze(1).broadcast_to((128, QI, H)),
                    Alu.mult,
                )
                nc.vector.tensor_reduce(
                    logits[:, :],
                    scratch[:, :].rearrange("p (q h) -> p q h", q=QI),
                    mybir.AxisListType.X,
                    Alu.add,
                )
                # softmax (no max subtraction: |logits| <= ||v||_1 ~ 9)
                e = spool.tile([128, QI], bf16, tag="e")
                esum = spool.tile([128, 1], f32, tag="esum")
                nc.scalar.activation(
                    e[:, :], logits[:, :], Act.Exp, accum_out=esum[:, :]
                )
                ps_S = ps_misc_pool.tile([128, 32], f32, tag="ps_misc")
                nc.tensor.matmul(
                    ps_S[:, 0:1], gmat[:, :], esum[:, :], start=True, stop=True
                )
                rS = spool.tile([128, 1], f32, tag="rS")
                nc.vector.reciprocal(rS[:, 0:1], ps_S[:, 0:1])
                ai = spool.tile([128, QI], bf16, tag="ai")
                nc.vector.tensor_scalar(ai[:, :], e[:, :], rS[:, 0:1], None, Alu.mult)
                # ediag[(q8,b), (qi,b')] = ai[(q8,b),qi] * mask
                ediag = spool.tile([128, QI * BS], bf16, tag="ediag")
                nc.vector.tensor_tensor(
                    ediag[:, :].rearrange("p (q j) -> p q j", q=QI),
                    ai[:, :].unsqueeze(2).broadcast_to((128, QI, BS)),
                    mask[:, :].rearrange("p (q j) -> p q j", q=QI),
                    Alu.mult,
                )
                # ct[b', h] accumulation over 7 masked-diagonal matmuls
                ps_ct = ps_ct_pool.tile([16, H], f32, tag="ps_ct")
                for qi in range(QI):
                    nc.tensor.matmul(
                        ps_ct[:, :],
                        ediag[:, qi * BS : (qi + 1) * BS],
                        qB[:, qi * H : (qi + 1) * H],
                        start=(qi == 0),
                        stop=(qi == 6),
                    )
                ct_sb = spool.tile([16, H], bf16, tag="ct_sb")
                nc.scalar.copy(ct_sb[:, :], ps_ct[:, :])
                ps_ctT = ps_misc_pool.tile([128, 32], bf16, tag="ps_misc")
                nc.tensor.transpose(ps_ctT[:, 0:16], ct_sb[:, :], i16[:, :])
                ctT = spool.tile([128, 16], bf16, tag="ctT")
                nc.vector.tensor_copy(ctT[:, :], ps_ctT[:, 0:16])

                # g = sigma(u0 @ Wg.T) transposed: (128,32) [j-half0 | j-half1]
                p_sT = pprojT[:, p * BS : (p + 1) * BS]
                ps_gT = ps_misc_pool.tile([128, 32], f32, tag="ps_misc")
                nc.tensor.matmul(
                    ps_gT[:, 0:16], wgt[:, 0:128], p_sT, start=True, stop=False
                )
                nc.tensor.matmul(
                    ps_gT[:, 0:16], wgt[:, 2 * H : 2 * H + 128], ctT[:, :],
                    start=False, stop=True,
                )
                nc.tensor.matmul(
                    ps_gT[:, 16:32], wgt[:, 128 : 2 * H], p_sT, start=True, stop=False
                )
                nc.tensor.matmul(
                    ps_gT[:, 16:32], wgt[:, 2 * H + 128 : 4 * H], ctT[:, :],
                    start=False, stop=True,
                )
                tg = spool.tile([128, 32], bf16, tag="tg")
                nc.scalar.activation(tg[:, :], ps_gT[:, :], Act.Tanh)
                gaff = spool.tile([128, 32], bf16, tag="gaff")
                nc.vector.tensor_scalar(
                    gaff[:, :], tg[:, :], 0.5, 0.5, Alu.mult, Alu.add
                )
                uTp = spool.tile([128, 16], bf16, tag="uTp")
                nc.vector.tensor_tensor(uTp[:, :], p_sT, gaff[:, 0:16], Alu.mult)
                uTc = spool.tile([128, 16], bf16, tag="uTc")
                nc.vector.tensor_tensor(uTc[:, :], ctT[:, :], gaff[:, 16:32], Alu.mult)

                # GRU matmuls: rz in its own bank; [gin | ghn] share a bank,
                # each accumulation group contiguous (bank-wipe hazard).
                ps_rz = ps_rz_pool.tile([16, 256], f32, tag="ps_rz")
                nc.tensor.matmul(
                    ps_rz[:, :], prevT[:, :], whht[:, 0:256],
                    start=True, stop=False, skip_group_check=True,
                )
                nc.tensor.matmul(
                    ps_rz[:, :], uTp[:, :], wiht[:, 0:256],
                    start=False, stop=False, skip_group_check=True,
                )
                nc.tensor.matmul(
                    ps_rz[:, :], uTc[:, :], wiht[:, 3 * H : 3 * H + 256],
                    start=False, stop=True, skip_group_check=True,
                )
                ps_gn = ps_gn_pool.tile([16, 256], f32, tag="ps_gn")
                nc.tensor.matmul(
                    ps_gn[:, 0:128], uTp[:, :], wiht[:, 256:384],
                    start=True, stop=False, skip_group_check=True,
                )
                nc.tensor.matmul(
                    ps_gn[:, 0:128], uTc[:, :], wiht[:, 3 * H + 256 : 6 * H],
                    start=False, stop=True, skip_group_check=True,
                )
                nc.tensor.matmul(
                    ps_gn[:, 128:256], prevT[:, :], whht[:, 256:384],
                    start=True, stop=True, skip_group_check=True,
                )
                girz_t = None
                if dbg_tap in ("girz", "girz_nogh"):
                    girz_t = spool.tile([16, 256], f32, tag="girz")
                    nc.vector.tensor_copy(girz_t[:, :], ps_rz[:, :])
                trz = spool.tile([16, 256], bf16, tag="trz")
                nc.scalar.activation(trz[:, :], ps_rz[:, :], Act.Tanh)
                rzaff = spool.tile([16, 256], bf16, tag="rzaff")
                nc.vector.tensor_scalar(
                    rzaff[:, :], trz[:, :], 0.5, 0.5, Alu.mult, Alu.add
                )
                tmp = spool.tile([16, H], bf16, tag="tmp")
                nc.vector.tensor_tensor(
                    tmp[:, :], rzaff[:, 0:128], ps_gn[:, 128:256], Alu.mult
                )
                npre = spool.tile([16, H], bf16, tag="npre")
                nc.vector.tensor_tensor(
                    npre[:, :], tmp[:, :], ps_gn[:, 0:128], Alu.add
                )
                nt = spool.tile([16, H], bf16, tag="nt")
                nc.scalar.activation(nt[:, :], npre[:, :], Act.Tanh)
                # new = n + z*(prev - n)   (f32)
                d1 = spool.tile([16, H], f32, tag="d1")
                nc.vector.tensor_tensor(d1[:, :], prev_b[:, :], nt[:, :], Alu.subtract)
                zd = spool.tile([16, H], f32, tag="zd")
                nc.vector.tensor_tensor(zd[:, :], d1[:, :], rzaff[:, 128:256], Alu.mult)
                new_b = spool.tile([16, H], f32, tag="prev_b")
                nc.vector.tensor_tensor(new_b[:, :], zd[:, :], nt[:, :], Alu.add)
                nc.vector.tensor_copy(ring[:, a, :], new_b[:, :])
                # prevT update via transpose
                ps_nT = ps_misc_pool.tile([128, 32], f32, tag="ps_misc")
                nc.tensor.transpose(ps_nT[:, 0:16], new_b[:, :], i16f[:, :])
                prevT_new = spool.tile([128, 16], bf16, tag="prevT")
                nc.vector.tensor_copy(prevT_new[:, :], ps_nT[:, 0:16])

                if dbg_d is not None:
                    _tap_tiles = {
                        "pl": pl_sb, "s": s_sb, "logits": logits, "ai": ai,
                        "ct": ct_sb, "tg": tg, "rzaff": rzaff, "nt": nt,
                        "ediag": ediag, "uTp": uTp, "uTc": uTc, "girz": girz_t, "girz_nogh": girz_t,
                    }
                    _t = _tap_tiles[dbg_tap]
                    _f = spool.tile(list(_t.shape), f32, tag="dbgf")
                    nc.vector.tensor_copy(_f[:, :], _t[:, :])
                    nc.sync.dma_start(dbg_d[p], _f[:, :])
                prev_b = new_b
                prevT = prevT_new

                if a == 7:
                    nc.sync.dma_start(
                        out_d[p - 7 : p + 1, :, :].rearrange("p b h -> b p h"),
                        ring[:, :, :],
                    )
                    ring = rpool.tile([16, 8, H], f32, tag="ring")

            for _pool in (ps_gn_pool, ps_rz_pool, ps_ct_pool, ps_misc_pool, ps_pl_pool, ps_s_pool):
                _pool.release()

    nc.compile()
    return nc


def _host_prep(inputs, n_steps=P):
    """Build per-core input maps."""
    import ml_dtypes

    bf = ml_dtypes.bfloat16
    pr = np.asarray(inputs["passage_repr"], np.float32)[:n_steps]
    qr = np.asarray(inputs["question_repr"], np.float32)
    WuQ = np.asarray(inputs["WuQ"], np.float32)
    WuP = np.asarray(inputs["WuP"], np.float32)
    WvP = np.asarray(inputs["WvP"], np.float32)
    v = np.asarray(inputs["vT"], np.float32)[0]
    Wg = np.asarray(inputs["Wg"], np.float32)
    W_ih = np.asarray(inputs["W_ih"], np.float32)
    W_hh = np.asarray(inputs["W_hh"], np.float32)

    wupt = np.ascontiguousarray(
        WuP.T.reshape(2, 128, H).astype(bf)
    )  # [c][k, h] = WuP[h, 128c+k]
    wuqt = np.ascontiguousarray(WuQ.T.reshape(2, 128, H).astype(bf))
    wvpt = np.ascontiguousarray(WvP.T.astype(bf))
    wgt = np.ascontiguousarray((Wg.T * 0.5).reshape(2, 128, 2 * H).astype(bf))
    wihT = W_ih.T.copy()
    wihT[:, :256] *= 0.5
    wiht = np.ascontiguousarray(wihT.reshape(2, 128, 3 * H).astype(bf))
    whhT = W_hh.T.copy()
    whhT[:, :256] *= 0.5
    whht = np.ascontiguousarray(whhT.astype(bf))

    i128 = np.eye(128, dtype=bf)
    i16 = np.eye(16, dtype=bf)
    i16f = np.eye(16, dtype=np.float32)
    pidx = np.arange(128)
    rmat = (np.arange(16)[:, None] == (pidx[None, :] % 16)).astype(bf)
    gmat = ((pidx[:, None] % 16) == (pidx[None, :] % 16)).astype(np.float32)
    bsel = ((pidx[:, None] % 16) == np.arange(16)[None, :]).astype(bf)
    mask = np.zeros((128, QI, BS), np.float32)
    for q8 in range(Q8):
        for b in range(BS):
            for qi in range(QI):
                if qi * 8 + q8 < Q:
                    mask[q8 * BS + b, qi, b] = 1.0
    mask = mask.reshape(128, QI * BS).astype(bf)
    vrep = np.broadcast_to(v, (128, H)).astype(bf)
    padq = np.broadcast_to(-30.0 * np.sign(v), (96, H)).astype(bf)

    shared = dict(
        wupt=wupt, wuqt=wuqt, wvpt=wvpt, wgt=wgt, wiht=wiht, whht=whht,
        i128=i128, i16=i16, i16f=i16f, rmat=rmat, gmat=gmat, bsel=bsel,
        mask=np.ascontiguousarray(mask), vrep=np.ascontiguousarray(vrep),
        padq=np.ascontiguousarray(padq),
    )
    in_maps = []
    for c in range(NC_):
        m = dict(shared)
        m["pr"] = np.ascontiguousarray(pr[:, c * BS : (c + 1) * BS, :])
        m["qr"] = np.ascontiguousarray(qr[:, c * BS : (c + 1) * BS, :])
        in_maps.append(m)
    return in_maps


def kernel(**inputs):
    global _compiled
    from concourse.bass_utils import run_bass_kernel_spmd

    if _compiled is None:
        _compiled = _build_nc(P)
    in_maps = _host_prep(inputs, P)
    res = run_bass_kernel_spmd(_compiled, in_maps, list(range(NC_)))
    outs = res.results
    full = np.concatenate([np.asarray(o["out"]) for o in outs], axis=1)
    return full.astype(np.float32)


# revision 22
# speedup vs baseline: 1.1719x; 1.1719x over previous
"""Trainium2 Bass kernel for nn_GatedElementBasedRNNLayer_Loop.

Structure (per core, batch shard b=16 of B=128 across 8 cores):
  - init: load inputs, project passage/question through WuP/WuQ on PE,
    lay question out as [(q8,b) partitions, (qi,h) free] (Q=50 padded to 56,
    pad rows get -30*sign(v) so their softmax weight is ~e^-9: negligible).
  - 400 sequential GRU-with-attention steps, all in bf16 (f32 state/psum):
      attention pre-act built in PSUM by PE (question feed + pl broadcast),
      tanh on ACT, v-dot via 7x tensor_tensor_reduce on DVE, softmax sums
      via ACT accum + a 0/1-matrix matmul, ct via masked-diagonal matmuls,
      GRU matmuls with small stationaries, sigmoid as 0.5*tanh(0.5x)+0.5
      (keeps ACT on one table set: exp+tanh).
"""
import sys
import numpy as np

sys.path.insert(0, "/opt/trn_rl_repo")

P, Q, B, I, H = 400, 50, 128, 256, 128
NC_ = 8
BS = B // NC_          # 16 per-core batch
Q8, QI = 8, 7          # q = qi*8 + q8, Q padded to 56
QPAD = Q8 * QI         # 56

_compiled = None


def _build_nc(n_steps=P, dbg_tap=None):
    import concourse.bass as bass
    import concourse.bacc as bacc
    from concourse import tile
    import concourse.mybir as mybir

    dt = mybir.dt
    f32, bf16 = dt.float32, dt.bfloat16
    Alu = mybir.AluOpType
    Act = mybir.ActivationFunctionType

    nc = bacc.Bacc("TRN2", target_bir_lowering=False, debug=False, num_devices=NC_)

    # ---- DRAM parameters (per-core) ----
    pr_d = nc.dram_tensor("pr", [n_steps, BS, I], bf16, kind="ExternalInput")
    qr_d = nc.dram_tensor("qr", [Q, BS, I], bf16, kind="ExternalInput")
    wupt_d = nc.dram_tensor("wupt", [2, 128, H], bf16, kind="ExternalInput")
    wuqt_d = nc.dram_tensor("wuqt", [2, 128, H], bf16, kind="ExternalInput")
    wvpt_d = nc.dram_tensor("wvpt", [H, H], bf16, kind="ExternalInput")
    wgt_d = nc.dram_tensor("wgt", [2, 128, 2 * H], bf16, kind="ExternalInput")
    wiht_d = nc.dram_tensor("wiht", [2, 128, 3 * H], bf16, kind="ExternalInput")
    whht_d = nc.dram_tensor("whht", [H, 3 * H], bf16, kind="ExternalInput")
    i128_d = nc.dram_tensor("i128", [128, 128], bf16, kind="ExternalInput")
    i16_d = nc.dram_tensor("i16", [16, 16], bf16, kind="ExternalInput")
    i16f_d = nc.dram_tensor("i16f", [16, 16], f32, kind="ExternalInput")
    rmat_d = nc.dram_tensor("rmat", [16, 128], bf16, kind="ExternalInput")
    gmat_d = nc.dram_tensor("gmat", [128, 128], f32, kind="ExternalInput")
    bsel_d = nc.dram_tensor("bsel", [128, 16], bf16, kind="ExternalInput")
    mask_d = nc.dram_tensor("mask", [128, QI * BS], bf16, kind="ExternalInput")
    vrep_d = nc.dram_tensor("vrep", [128, H], bf16, kind="ExternalInput")
    padq_d = nc.dram_tensor("padq", [96, H], bf16, kind="ExternalInput")
    out_d = nc.dram_tensor("out", [n_steps, BS, H], bf16, kind="ExternalOutput")
    _tap_shapes = {
        "pl": [16, H], "s": [128, QI * H], "logits": [128, QI], "ai": [128, QI],
        "ct": [16, H], "tg": [128, 32], "rzaff": [16, 256], "nt": [16, H],
        "ediag": [128, QI * BS], "uTp": [128, 16], "uTc": [128, 16], "girz": [16, 256], "girz_nogh": [16, 256],
    }
    dbg_d = None
    if dbg_tap is not None:
        dbg_d = nc.dram_tensor(
            "dbg", [n_steps] + _tap_shapes[dbg_tap], f32, kind="ExternalOutput"
        )

    with tile.TileContext(nc) as tc:
        with (
            tc.tile_pool(name="const", bufs=1) as cpool,
            tc.tile_pool(name="work", bufs=2) as wpool,
            tc.tile_pool(name="small", bufs=3) as spool,
            tc.tile_pool(name="ring", bufs=2) as rpool,
        ):
            # ---------------- constants to SBUF ----------------
            i128 = cpool.tile([128, 128], bf16, tag="i128")
            nc.sync.dma_start(i128[:, :], i128_d[:, :])
            i16 = cpool.tile([16, 16], bf16, tag="i16")
            nc.sync.dma_start(i16[:, :], i16_d[:, :])
            i16f = cpool.tile([16, 16], f32, tag="i16f")
            nc.sync.dma_start(i16f[:, :], i16f_d[:, :])
            rmat = cpool.tile([16, 128], bf16, tag="rmat")
            nc.sync.dma_start(rmat[:, :], rmat_d[:, :])
            gmat = cpool.tile([128, 128], f32, tag="gmat")
            nc.sync.dma_start(gmat[:, :], gmat_d[:, :])
            mask = cpool.tile([128, QI * BS], bf16, tag="mask")
            nc.sync.dma_start(mask[:, :], mask_d[:, :])
            vrep = cpool.tile([128, H], bf16, tag="vrep")
            nc.sync.dma_start(vrep[:, :], vrep_d[:, :])
            wvpt = cpool.tile([H, H], bf16, tag="wvpt")
            nc.sync.dma_start(wvpt[:, :], wvpt_d[:, :])
            whht = cpool.tile([H, 3 * H], bf16, tag="whht")
            nc.sync.dma_start(whht[:, :], whht_d[:, :])
            wgt = cpool.tile([128, 2 * 2 * H], bf16, tag="wgt")  # chunks side by side
            nc.sync.dma_start(wgt[:, 0 : 2 * H], wgt_d[0, :, :])
            nc.sync.dma_start(wgt[:, 2 * H : 4 * H], wgt_d[1, :, :])
            wiht = cpool.tile([128, 2 * 3 * H], bf16, tag="wiht")
            nc.sync.dma_start(wiht[:, 0 : 3 * H], wiht_d[0, :, :])
            nc.sync.dma_start(wiht[:, 3 * H : 6 * H], wiht_d[1, :, :])
            wupt = cpool.tile([128, 2 * H], bf16, tag="wupt")
            nc.sync.dma_start(wupt[:, 0:H], wupt_d[0, :, :])
            nc.sync.dma_start(wupt[:, H : 2 * H], wupt_d[1, :, :])
            wuqt = cpool.tile([128, 2 * H], bf16, tag="wuqt")
            nc.sync.dma_start(wuqt[:, 0:H], wuqt_d[0, :, :])
            nc.sync.dma_start(wuqt[:, H : 2 * H], wuqt_d[1, :, :])

            # persistent step data
            qB = cpool.tile([128, QI * H], bf16, tag="qB")          # question layout B'
            pprojT = cpool.tile([128, n_steps * BS], bf16, tag="pprojT")   # [h, (p,b)]

            # ---------------- input load + projections ----------------
            with (
                tc.tile_pool(name="init", bufs=1) as ipool,
                tc.tile_pool(name="ps_init", bufs=2, space="PSUM") as ps_ipool,
            ):
                # question: load, cast, project, transpose into qB
                qrb = ipool.tile([128, 2, Q * BS], bf16, tag="qrb")
                for c in range(2):
                    nc.sync.dma_start(
                        qrb[:, c, :],
                        qr_d[:, :, 128 * c : 128 * (c + 1)].rearrange(
                            "q b i -> i (q b)"
                        ),
                    )
                for qb in range(QI):
                    ncols = min(8, Q - 8 * qb) * BS  # 128, last block 32
                    ps_q = ps_ipool.tile([128, 128], f32, tag="psq")
                    for c in range(2):
                        nc.tensor.matmul(
                            ps_q[:, 0:ncols],
                            wuqt[:, H * c : H * (c + 1)],
                            qrb[:, c, 8 * qb * BS : 8 * qb * BS + ncols],
                            start=(c == 0),
                            stop=(c == 1),
                        )
                    qp_sb = ipool.tile([128, 128], bf16, tag="qp_sb")
                    nc.scalar.copy(qp_sb[:, 0:ncols], ps_q[:, 0:ncols])
                    ps_qT = ps_ipool.tile([128, 128], bf16, tag="psqT")
                    nc.tensor.transpose(
                        ps_qT[0:ncols, :], qp_sb[:, 0:ncols], i128[:, :]
                    )
                    nc.scalar.copy(
                        qB[0:ncols, H * qb : H * (qb + 1)], ps_qT[0:ncols, :]
                    )
                # pad rows of last q-block
                nc.sync.dma_start(qB[32:128, H * 6 : H * 7], padq_d[:, :])

                # passage: process in column-chunks of 80 p to bound SBUF
                CH = min(80, n_steps)
                for chunk in range(n_steps // CH):
                    p0 = chunk * CH
                    prb = ipool.tile([128, 2, CH * BS], bf16, tag="prb")
                    for c in range(2):
                        nc.sync.dma_start(
                            prb[:, c, :],
                            pr_d[p0 : p0 + CH, :, 128 * c : 128 * (c + 1)].rearrange(
                                "p b i -> i (p b)"
                            ),
                        )
                    for pb in range(CH // 8):
                        ps_p = ps_ipool.tile([128, 128], f32, tag="psp")
                        for c in range(2):
                            nc.tensor.matmul(
                                ps_p[:, :],
                                wupt[:, H * c : H * (c + 1)],
                                prb[:, c, 128 * pb : 128 * (pb + 1)],
                                start=(c == 0),
                                stop=(c == 1),
                            )
                        col = (p0 // 8 + pb) * 128
                        nc.scalar.copy(pprojT[:, col : col + 128], ps_p[:, :])

            # ---------------- state init ----------------
            ps_s_pool = tc.alloc_tile_pool(name="ps_s", bufs=1, space="PSUM")
            ps_pl_pool = tc.alloc_tile_pool(name="ps_pl", bufs=1, space="PSUM")
            ps_misc_pool = tc.alloc_tile_pool(name="ps_misc", bufs=2, space="PSUM")
            ps_ct_pool = tc.alloc_tile_pool(name="ps_ct", bufs=1, space="PSUM")
            ps_rz_pool = tc.alloc_tile_pool(name="ps_rz", bufs=1, space="PSUM")
            ps_gn_pool = tc.alloc_tile_pool(name="ps_gn", bufs=1, space="PSUM")
            prevT = spool.tile([128, 16], bf16, tag="prevT")
            nc.vector.memset(prevT[:, :], 0.0)
            prev_b = spool.tile([16, H], f32, tag="prev_b")
            nc.vector.memset(prev_b[:, :], 0.0)
            ring = rpool.tile([16, 8, H], bf16, tag="ring")

            # ---------------- main loop ----------------
            for p in range(n_steps):
                a = p % 8
                pblk = p // 8
                # pl = prev @ WvP.T + pproj[p]  -> psum (16,128)
                ps_pl = ps_pl_pool.tile([16, H], f32, tag="ps_pl")
                nc.tensor.matmul(
                    ps_pl[:, :], prevT[:, :], wvpt[:, :], start=True, stop=False
                )
                nc.tensor.matmul(
                    ps_pl[:, :],
                    pprojT[:, p * BS : (p + 1) * BS],
                    i128[:, :],
                    start=False,
                    stop=True,
                )
                pl_sb = spool.tile([16, H], bf16, tag="pl_sb")
                nc.scalar.copy(pl_sb[:, :], ps_pl[:, :])

                # psum_s = question + broadcast(pl)
                ps_s = ps_s_pool.tile([128, QI * H], f32, tag="ps_s")
                nc.tensor.matmul(
                    ps_s[:, 0:512], i128[:, :], qB[:, 0:512],
                    start=True, stop=False, skip_group_check=True,
                )
                nc.tensor.matmul(
                    ps_s[:, 512:896], i128[:, :], qB[:, 512:896],
                    start=True, stop=False, skip_group_check=True,
                )
                for qi in range(QI):
                    nc.tensor.matmul(
                        ps_s[:, qi * H : (qi + 1) * H],
                        rmat[:, :],
                        pl_sb[:, :],
                        start=False,
                        stop=(qi == 3 or qi == 6),
                        skip_group_check=True,
                    )
                s_sb = wpool.tile([128, QI * H], bf16, tag="s_sb")
                nc.scalar.activation(s_sb[:, :], ps_s[:, :], Act.Tanh)

                # logits[(q8,b), qi] = sum_h s*v
                logits = spool.tile([128, QI], f32, tag="logits")
                scratch = wpool.tile([128, QI * H], bf16, tag="scratch")
                nc.vector.tensor_tensor(
                    scratch[:, :].rearrange("p (q h) -> p q h", q=QI),
                    s_sb[:, :].rearrange("p (q h) -> p q h", q=QI),
                    vrep[:, :].unsqueeze(1).broadcast_to((128, QI, H)),
                    Alu.mult,
                )
                nc.vector.tensor_reduce(
                    logits[:, :],
                    scratch[:, :].rearrange("p (q h) -> p q h", q=QI),
                    mybir.AxisListType.X,
                    Alu.add,
                )
                # softmax (no max subtraction: |logits| <= ||v||_1 ~ 9)
                e = spool.tile([128, QI], bf16, tag="e")
                esum = spool.tile([128, 1], f32, tag="esum")
                nc.scalar.activation(
                    e[:, :], logits[:, :], Act.Exp, accum_out=esum[:, :]
                )
                ps_S = ps_misc_pool.tile([128, 32], f32, tag="ps_misc")
                nc.tensor.matmul(
                    ps_S[:, 0:1], gmat[:, :], esum[:, :], start=True, stop=True
                )
                rS = spool.tile([128, 1], f32, tag="rS")
                nc.vector.reciprocal(rS[:, 0:1], ps_S[:, 0:1])
                ai = spool.tile([128, QI], bf16, tag="ai")
                nc.vector.tensor_scalar(ai[:, :], e[:, :], rS[:, 0:1], None, Alu.mult)
                # ediag[(q8,b), (qi,b')] = ai[(q8,b),qi] * mask
                ediag = spool.tile([128, QI * BS], bf16, tag="ediag")
                nc.vector.tensor_tensor(
                    ediag[:, :].rearrange("p (q j) -> p q j", q=QI),
                    ai[:, :].unsqueeze(2).broadcast_to((128, QI, BS)),
                    mask[:, :].rearrange("p (q j) -> p q j", q=QI),
                    Alu.mult,
                )
                # ct[b', h] accumulation over 7 masked-diagonal matmuls
                ps_ct = ps_ct_pool.tile([16, H], f32, tag="ps_ct")
                for qi in range(QI):
                    nc.tensor.matmul(
                        ps_ct[:, :],
                        ediag[:, qi * BS : (qi + 1) * BS],
                        qB[:, qi * H : (qi + 1) * H],
                        start=(qi == 0),
                        stop=(qi == 6),
                    )
                ct_sb = spool.tile([16, H], bf16, tag="ct_sb")
                nc.scalar.copy(ct_sb[:, :], ps_ct[:, :])
                ps_ctT = ps_misc_pool.tile([128, 32], bf16, tag="ps_misc")
                nc.tensor.transpose(ps_ctT[:, 0:16], ct_sb[:, :], i16[:, :])
                ctT = spool.tile([128, 16], bf16, tag="ctT")
                nc.vector.tensor_copy(ctT[:, :], ps_ctT[:, 0:16])

                # g = sigma(u0 @ Wg.T) transposed: (128,32) [j-half0 | j-half1]
                p_sT = pprojT[:, p * BS : (p + 1) * BS]
                ps_gT = ps_misc_pool.tile([128, 32], f32, tag="ps_misc")
                nc.tensor.matmul(
                    ps_gT[:, 0:16], wgt[:, 0:128], p_sT, start=True, stop=False
                )
                nc.tensor.matmul(
                    ps_gT[:, 0:16], wgt[:, 2 * H : 2 * H + 128], ctT[:, :],
                    start=False, stop=True,
                )
                nc.tensor.matmul(
                    ps_gT[:, 16:32], wgt[:, 128 : 2 * H], p_sT, start=True, stop=False
                )
                nc.tensor.matmul(
                    ps_gT[:, 16:32], wgt[:, 2 * H + 128 : 4 * H], ctT[:, :],
                    start=False, stop=True,
                )
                tg = spool.tile([128, 32], bf16, tag="tg")
                nc.scalar.activation(tg[:, :], ps_gT[:, :], Act.Tanh)
                gaff = spool.tile([128, 32], bf16, tag="gaff")
                nc.vector.tensor_scalar(
                    gaff[:, :], tg[:, :], 0.5, 0.5, Alu.mult, Alu.add
                )
                uTp = spool.tile([128, 16], bf16, tag="uTp")
                nc.vector.tensor_tensor(uTp[:, :], p_sT, gaff[:, 0:16], Alu.mult)
                uTc = spool.tile([128, 16], bf16, tag="uTc")
                nc.vector.tensor_tensor(uTc[:, :], ctT[:, :], gaff[:, 16:32], Alu.mult)

                # GRU matmuls: rz in its own bank; [gin | ghn] share a bank,
                # each accumulation group contiguous (bank-wipe hazard).
                ps_rz = ps_rz_pool.tile([16, 256], f32, tag="ps_rz")
                nc.tensor.matmul(
                    ps_rz[:, :], prevT[:, :], whht[:, 0:256],
                    start=True, stop=False, skip_group_check=True,
                )
                nc.tensor.matmul(
                    ps_rz[:, :], uTp[:, :], wiht[:, 0:256],
                    start=False, stop=False, skip_group_check=True,
                )
                nc.tensor.matmul(
                    ps_rz[:, :], uTc[:, :], wiht[:, 3 * H : 3 * H + 256],
                    start=False, stop=True, skip_group_check=True,
                )
                ps_gn = ps_gn_pool.tile([16, 256], f32, tag="ps_gn")
                nc.tensor.matmul(
                    ps_gn[:, 0:128], uTp[:, :], wiht[:, 256:384],
                    start=True, stop=False, skip_group_check=True,
                )
                nc.tensor.matmul(
                    ps_gn[:, 0:128], uTc[:, :], wiht[:, 3 * H + 256 : 6 * H],
                    start=False, stop=True, skip_group_check=True,
                )
                nc.tensor.matmul(
                    ps_gn[:, 128:256], prevT[:, :], whht[:, 256:384],
                    start=True, stop=True, skip_group_check=True,
                )
                girz_t = None
                if dbg_tap in ("girz", "girz_nogh"):
                    girz_t = spool.tile([16, 256], f32, tag="girz")
                    nc.vector.tensor_copy(girz_t[:, :], ps_rz[:, :])
                trz = spool.tile([16, 256], bf16, tag="trz")
                nc.scalar.activation(trz[:, :], ps_rz[:, :], Act.Tanh)
                rzaff = spool.tile([16, 256], bf16, tag="rzaff")
                nc.vector.tensor_scalar(
                    rzaff[:, :], trz[:, :], 0.5, 0.5, Alu.mult, Alu.add
                )
                tmp = spool.tile([16, H], bf16, tag="tmp")
                nc.vector.tensor_tensor(
                    tmp[:, :], rzaff[:, 0:128], ps_gn[:, 128:256], Alu.mult
                )
                npre = spool.tile([16, H], bf16, tag="npre")
                nc.vector.tensor_tensor(
                    npre[:, :], tmp[:, :], ps_gn[:, 0:128], Alu.add
                )
                nt = spool.tile([16, H], bf16, tag="nt")
                nc.scalar.activation(nt[:, :], npre[:, :], Act.Tanh)
                # new = n + z*(prev - n)   (f32)
                d1 = spool.tile([16, H], f32, tag="d1")
                nc.vector.tensor_tensor(d1[:, :], prev_b[:, :], nt[:, :], Alu.subtract)
                zd = spool.tile([16, H], f32, tag="zd")
                nc.vector.tensor_tensor(zd[:, :], d1[:, :], rzaff[:, 128:256], Alu.mult)
                new_b = spool.tile([16, H], f32, tag="prev_b")
                nc.vector.tensor_tensor(new_b[:, :], zd[:, :], nt[:, :], Alu.add)
                nc.vector.tensor_copy(ring[:, a, :], new_b[:, :])
                # prevT update via transpose
                ps_nT = ps_misc_pool.tile([128, 32], f32, tag="ps_misc")
                nc.tensor.transpose(ps_nT[:, 0:16], new_b[:, :], i16f[:, :])
                prevT_new = spool.tile([128, 16], bf16, tag="prevT")
                nc.vector.tensor_copy(prevT_new[:, :], ps_nT[:, 0:16])

                if dbg_d is not None:
                    _tap_tiles = {
                        "pl": pl_sb, "s": s_sb, "logits": logits, "ai": ai,
                        "ct": ct_sb, "tg": tg, "rzaff": rzaff, "nt": nt,
                        "ediag": ediag, "uTp": uTp, "uTc": uTc, "girz": girz_t, "girz_nogh": girz_t,
                    }
                    _t = _tap_tiles[dbg_tap]
                    _f = spool.tile(list(_t.shape), f32, tag="dbgf")
                    nc.vector.tensor_copy(_f[:, :], _t[:, :])
                    nc.sync.dma_start(dbg_d[p], _f[:, :])
                prev_b = new_b
                prevT = prevT_new

                if a == 7:
                    nc.sync.dma_start(
                        out_d[p - 7 : p + 1, :, :].rearrange("p b h -> b p h"),
                        ring[:, :, :],
                    )
                    ring = rpool.tile([16, 8, H], bf16, tag="ring")

            for _pool in (ps_gn_pool, ps_rz_pool, ps_ct_pool, ps_misc_pool, ps_pl_pool, ps_s_pool):
                _pool.release()

    nc.compile()
    return nc


def _host_prep(inputs, n_steps=P):
    """Build per-core input maps."""
    import ml_dtypes

    bf = ml_dtypes.bfloat16
    pr = np.asarray(inputs["passage_repr"], np.float32)[:n_steps].astype(bf)
    qr = np.asarray(inputs["question_repr"], np.float32).astype(bf)
    WuQ = np.asarray(inputs["WuQ"], np.float32)
    WuP = np.asarray(inputs["WuP"], np.float32)
    WvP = np.asarray(inputs["WvP"], np.float32)
    v = np.asarray(inputs["vT"], np.float32)[0]
    Wg = np.asarray(inputs["Wg"], np.float32)
    W_ih = np.asarray(inputs["W_ih"], np.float32)
    W_hh = np.asarray(inputs["W_hh"], np.float32)

    wupt = np.ascontiguousarray(
        WuP.T.reshape(2, 128, H).astype(bf)
    )  # [c][k, h] = WuP[h, 128c+k]
    wuqt = np.ascontiguousarray(WuQ.T.reshape(2, 128, H).astype(bf))
    wvpt = np.ascontiguousarray(WvP.T.astype(bf))
    wgt = np.ascontiguousarray((Wg.T * 0.5).reshape(2, 128, 2 * H).astype(bf))
    wihT = W_ih.T.copy()
    wihT[:, :256] *= 0.5
    wiht = np.ascontiguousarray(wihT.reshape(2, 128, 3 * H).astype(bf))
    whhT = W_hh.T.copy()
    whhT[:, :256] *= 0.5
    whht = np.ascontiguousarray(whhT.astype(bf))

    i128 = np.eye(128, dtype=bf)
    i16 = np.eye(16, dtype=bf)
    i16f = np.eye(16, dtype=np.float32)
    pidx = np.arange(128)
    rmat = (np.arange(16)[:, None] == (pidx[None, :] % 16)).astype(bf)
    gmat = ((pidx[:, None] % 16) == (pidx[None, :] % 16)).astype(np.float32)
    bsel = ((pidx[:, None] % 16) == np.arange(16)[None, :]).astype(bf)
    mask = np.zeros((128, QI, BS), np.float32)
    for q8 in range(Q8):
        for b in range(BS):
            for qi in range(QI):
                if qi * 8 + q8 < Q:
                    mask[q8 * BS + b, qi, b] = 1.0
    mask = mask.reshape(128, QI * BS).astype(bf)
    vrep = np.broadcast_to(v, (128, H)).astype(bf)
    padq = np.broadcast_to(-30.0 * np.sign(v), (96, H)).astype(bf)

    shared = dict(
        wupt=wupt, wuqt=wuqt, wvpt=wvpt, wgt=wgt, wiht=wiht, whht=whht,
        i128=i128, i16=i16, i16f=i16f, rmat=rmat, gmat=gmat, bsel=bsel,
        mask=np.ascontiguousarray(mask), vrep=np.ascontiguousarray(vrep),
        padq=np.ascontiguousarray(padq),
    )
    in_maps = []
    for c in range(NC_):
        m = dict(shared)
        m["pr"] = np.ascontiguousarray(pr[:, c * BS : (c + 1) * BS, :])
        m["qr"] = np.ascontiguousarray(qr[:, c * BS : (c + 1) * BS, :])
        in_maps.append(m)
    return in_maps


_runner = None
_outbufs = None


def _make_runner(nc):
    import jax
    import jax.numpy as jnp
    from jax.sharding import Mesh, PartitionSpec
    from jax.experimental.shard_map import shard_map
    import concourse.mybir as mybir
    from concourse import bass2jax

    bass2jax.install_neuronx_cc_hook()
    partition_name = nc.partition_id_tensor.name if nc.partition_id_tensor else None
    in_names, out_names, out_avals = [], [], []
    for alloc in nc.m.functions[0].allocations:
        if not isinstance(alloc, mybir.MemoryLocationSet):
            continue
        name = alloc.memorylocations[0].name
        if alloc.kind == "ExternalInput":
            if name != partition_name:
                in_names.append(name)
        elif alloc.kind == "ExternalOutput":
            shape = tuple(alloc.tensor_shape)
            dtype = mybir.dt.np(alloc.dtype)
            out_names.append(name)
            out_avals.append(jax.core.ShapedArray(shape, dtype))
    n_params = len(in_names)
    all_names = in_names + out_names
    if partition_name is not None:
        all_names = all_names + [partition_name]

    def _body(*args):
        operands = list(args)
        if partition_name is not None:
            operands.append(bass2jax.partition_id_tensor())
        outs = bass2jax._bass_exec_p.bind(
            *operands,
            out_avals=tuple(out_avals),
            in_names=tuple(all_names),
            out_names=tuple(out_names),
            lowering_input_output_aliases=(),
            sim_require_finite=True,
            sim_require_nnan=True,
            nc=nc,
        )
        return tuple(outs)

    devices = jax.devices()[:NC_]
    mesh = Mesh(np.asarray(devices), ("core",))
    in_specs = (PartitionSpec("core"),) * (n_params + len(out_names))
    out_specs = (PartitionSpec("core"),) * len(out_names)
    donate = tuple(range(n_params, n_params + len(out_names)))
    jitted = jax.jit(
        shard_map(_body, mesh=mesh, in_specs=in_specs, out_specs=out_specs,
                  check_rep=False),
        donate_argnums=donate,
        keep_unused=True,
    )
    return jitted, in_names, out_names, out_avals


def kernel(**inputs):
    global _compiled, _runner, _outbufs
    if _compiled is None:
        _compiled = _build_nc(P)
    if _runner is None:
        _runner = _make_runner(_compiled)
    jitted, in_names, out_names, out_avals = _runner
    in_maps = _host_prep(inputs, P)
    concat_in = [
        np.concatenate([in_maps[c][name] for c in range(NC_)], axis=0)
        for name in in_names
    ]
    if _outbufs is None:
        _outbufs = [
            np.zeros((NC_ * a.shape[0], *a.shape[1:]), a.dtype) for a in out_avals
        ]
    out_arrs = jitted(*concat_in, *_outbufs)
    _outbufs = list(out_arrs)
    full_shard = np.asarray(out_arrs[out_names.index("out")])  # (NC*P, BS, H) bf16
    full = full_shard.reshape(NC_, P, BS, H).transpose(1, 0, 2, 3).reshape(P, B, H)
    return full.astype(np.float32)
